# revision 17
# baseline (speedup 1.0000x reference)
"""CLAHE (RGB->Lab, per-tile hist equalization, bilinear LUT interp, Lab->RGB)
on 8 trn2 NeuronCores. Self-contained: hardcodes shapes for x [3, 2048, 2048] f32.

Sharding: core c handles image rows [256c, 256c+256). Each core computes its
own tile row's histograms/LUTs on-device; LUTs are exchanged with an on-chip
AllGather; the bilinear interpolation needs only the +-1 tile-row LUTs
(selected via host-provided per-core offsets).
"""

from contextlib import ExitStack

import numpy as np

import concourse.bass as bass
import concourse.bacc as bacc
import concourse.mybir as mybir
import concourse.tile as tile
from concourse.bass_utils import run_bass_kernel_spmd

F32 = mybir.dt.float32
BF16 = mybir.dt.bfloat16
U16 = mybir.dt.uint16
I32 = mybir.dt.int32
I8 = mybir.dt.int8
AL = mybir.AluOpType
ACT = mybir.ActivationFunctionType
AX = mybir.AxisListType

H = W = 2048
NB = 256        # bins
TRW = 256       # rows per core
NCORES = 8
CLIPV = 2560.0  # CLIP_LIMIT * tile_area / NB = 10*65536/256
LSCALE = 255.0 / 65536.0
EPS = 0.008856

_M_RGB2XYZ = np.array([[0.412453, 0.357580, 0.180423],
                       [0.212671, 0.715160, 0.072169],
                       [0.019334, 0.119193, 0.950227]], np.float64)
_M_XYZ2RGB = np.array([[3.240479, -1.537150, -0.498535],
                       [-0.969256, 1.875992, 0.041556],
                       [0.055648, -0.204043, 1.057311]], np.float64)
_WP = np.array([0.950456, 1.0, 1.088754], np.float64)
M_FWD = (_M_RGB2XYZ / _WP[:, None]).astype(np.float32)   # xyz' = M_FWD @ rgb
M_BWD = (_M_XYZ2RGB * _WP[None, :]).astype(np.float32)   # rgb = M_BWD @ f^3

# column spans with constant (tc0, tc1); tc1 = min(tc0+1, 7)
SPANS = []
for _t in range(8):
    _lo = 0 if _t == 0 else 384 + 256 * (_t - 1)
    _hi = min(384 + 256 * _t, 2048)
    SPANS.append((_lo, _hi, _t, min(_t + 1, 7)))


def _forward_color(nc, tmp, keep):
    """rgb tiles -> Lq(i32), vb(bf16), ap, bp tiles in `keep` (one block)."""
    rgb_t = keep.pop("rgb")
    xyz = []
    for d in range(3):
        t = tmp.tile([128, 2048], F32, tag="tmp")
        nc.vector.tensor_scalar(t[:], rgb_t[0][:], float(M_FWD[d][0]), None, AL.mult)
        nc.vector.scalar_tensor_tensor(t[:], rgb_t[1][:], float(M_FWD[d][1]), t[:],
                                       AL.mult, AL.add)
        nc.vector.scalar_tensor_tensor(t[:], rgb_t[2][:], float(M_FWD[d][2]), t[:],
                                       AL.mult, AL.add)
        xyz.append(t)
    fs = []
    for d in range(3):
        x = xyz[d]
        ln = tmp.tile([128, 2048], F32, tag="tmp")
        nc.scalar.activation(ln[:], x[:], ACT.Ln)
        cb = tmp.tile([128, 2048], F32, tag="tmp")
        nc.scalar.activation(cb[:], ln[:], ACT.Exp, scale=1.0 / 3.0)
        if d == 1:
            # Newton polish: cb = (2*cb + x/cb^2)/3
            y2 = tmp.tile([128, 2048], F32, tag="tmp")
            nc.vector.tensor_tensor(y2[:], cb[:], cb[:], AL.mult)
            rc = tmp.tile([128, 2048], F32, tag="tmp")
            nc.vector.reciprocal(rc[:], y2[:])
            nc.vector.tensor_tensor(rc[:], rc[:], x[:], AL.mult)
            nc.vector.scalar_tensor_tensor(cb[:], cb[:], 2.0, rc[:], AL.mult, AL.add)
            nc.vector.tensor_scalar(cb[:], cb[:], 1.0 / 3.0, None, AL.mult)
        lin = tmp.tile([128, 2048], F32, tag="tmp")
        nc.vector.tensor_scalar(lin[:], x[:], 7.787, 16.0 / 116.0, AL.mult, AL.add)
        msk = tmp.tile([128, 2048], I8, tag="tmp8", bufs=2)
        nc.vector.tensor_scalar(msk[:], x[:], EPS, None, AL.is_gt)
        nc.vector.select(x[:], msk[:], cb[:], lin[:])
        fs.append(x)
    l8 = tmp.tile([128, 2048], F32, tag="tmp")
    nc.vector.tensor_scalar(l8[:], fs[1][:], 116.0 * 2.55, -16.0 * 2.55 + 0.5,
                            AL.mult, AL.add)
    nc.vector.tensor_scalar(l8[:], l8[:], 0.5, 255.5, AL.max, AL.min)
    nc.vector.tensor_copy(keep["Lq"][:], l8[:])   # trunc(sim)/RNE(hw) -> int
    nc.vector.tensor_scalar(keep["Lq"][:], keep["Lq"][:], 255, None, AL.min)
    nc.vector.tensor_copy(keep["vb"][:], keep["Lq"][:])  # i32 -> bf16
    nc.vector.tensor_tensor(keep["ap"][:], fs[0][:], fs[1][:], AL.subtract)
    nc.vector.tensor_tensor(keep["bp"][:], fs[1][:], fs[2][:], AL.subtract)


def build_kernel(n_cores: int = NCORES, use_collective: bool = True,
                 n_w: int = 256, dbg: bool = False):
    es = ExitStack()
    nc = bacc.Bacc(None, target_bir_lowering=False, debug=False)

    rgb_in = nc.dram_tensor("rgb", [3, TRW, W], F32, kind="ExternalInput")
    wy_in = nc.dram_tensor("wy", [2, 128], F32, kind="ExternalInput")
    wx_in = nc.dram_tensor("wx", [1, W], F32, kind="ExternalInput")
    sel_in = nc.dram_tensor("sel", [1, 3], I32, kind="ExternalInput")
    out_t = nc.dram_tensor("out", [3, TRW, W], F32, kind="ExternalOutput")
    if dbg:
        dbg_lq = nc.dram_tensor("dbg_lq", [2, 128, 2048], I32, kind="ExternalOutput")
        dbg_hl = nc.dram_tensor("dbg_hl", [8, 256], F32, kind="ExternalOutput")
        dbg_lut = nc.dram_tensor("dbg_lut", [8, 256], F32, kind="ExternalOutput")
        dbg_acc = nc.dram_tensor("dbg_acc", [2, 128, 2048], I32, kind="ExternalOutput")
        dbg_vtp = nc.dram_tensor("dbg_vtp", [128, 2048], I32, kind="ExternalOutput")
        dbg_hs = nc.dram_tensor("dbg_hs", [128, 128], F32, kind="ExternalOutput")

    selnp = np.zeros((128, 16), np.float32)
    selnp[np.arange(128), np.arange(128) % 16] = 1.0
    sel_const = nc.inline_tensor(selnp, name="selc")
    mbnp = np.zeros((128, 8), np.float32)
    mbnp[np.arange(128), np.arange(128) // 16] = 1.0
    mb_const = nc.inline_tensor(mbnp, name="mbc")

    with tile.TileContext(nc) as tc:
        with (
            tc.tile_pool(name="pers", bufs=1) as pers,
            tc.tile_pool(name="tmp", bufs=7) as tmp,
            tc.tile_pool(name="ps", bufs=1, space="PSUM") as ps,
            tc.tile_pool(name="dr", bufs=1, space="DRAM") as dr,
        ):
            keep = [
                {"Lq": pers.tile([128, 2048], I32, tag=f"Lq{b}", name=f"Lq{b}"),
                 "vb": pers.tile([128, 2048], BF16, tag=f"vb{b}", name=f"vb{b}"),
                 "ap": pers.tile([128, 2048], F32, tag=f"apt{b}", name=f"apt{b}"),
                 "bp": pers.tile([128, 2048], F32, tag=f"bpt{b}", name=f"bpt{b}")}
                for b in range(2)
            ]

            selc_sb = pers.tile([128, 16], F32, tag="selc")
            nc.sync.dma_start(selc_sb[:], sel_const[:])
            mb_sb = pers.tile([128, 8], F32, tag="mbc")
            nc.sync.dma_start(mb_sb[:], mb_const[:])

            # ---------- phase 1: color forward per block ----------
            for b in range(2):
                rgb_t = []
                for ch in range(3):
                    rt = tmp.tile([128, 2048], F32, tag="tmp")
                    nc.sync.dma_start(rt[:], rgb_in[ch, 128 * b:128 * (b + 1), :])
                    rgb_t.append(rt)
                keep[b]["rgb"] = rgb_t
                _forward_color(nc, tmp, keep[b])

            # ---------- phase 2: histogram (PE nibble outer products) ------
            hpsum = [ps.tile([128, 128], F32, tag=f"hp{t}", name=f"hp{t}")[:, :]
                     for t in range(8)]
            first = [True] * 8
            for b in range(2):
                hi_i = tmp.tile([128, 2048], I32, tag="tmp")
                nc.vector.tensor_scalar(hi_i[:], keep[b]["Lq"][:], 4, None,
                                        AL.logical_shift_right)
                lo_i = tmp.tile([128, 2048], I32, tag="tmp")
                nc.vector.tensor_scalar(lo_i[:], keep[b]["Lq"][:], 15, None,
                                        AL.bitwise_and)
                hi_b = tmp.tile([128, 2048], BF16, tag="tmphb", bufs=1)
                nc.vector.tensor_copy(hi_b[:], hi_i[:])
                lo_b = tmp.tile([128, 2048], BF16, tag="tmplb", bufs=1)
                nc.vector.tensor_copy(lo_b[:], lo_i[:])
                for chunk in range(8):  # 256 cols each
                    c0 = 256 * chunk
                    mh = tmp.tile([128, 16, 256], BF16, tag="tmpmh", bufs=1)
                    ml = tmp.tile([128, 16, 256], BF16, tag="tmpml", bufs=1)
                    for j in range(16):
                        nc.vector.tensor_scalar(mh[:, j, :], hi_b[:, c0:c0 + 256],
                                                float(j), None, AL.is_equal)
                        nc.vector.tensor_scalar(ml[:, j, :], lo_b[:, c0:c0 + 256],
                                                float(j), None, AL.is_equal)
                    mhc = tmp.tile([128, 256, 16], BF16, tag="tmpmhc", bufs=1)
                    nc.vector.tensor_copy(mhc[:],
                                          mh[:, :, :].rearrange("p j c -> p c j"))
                    mlc = tmp.tile([128, 256, 16], BF16, tag="tmpmlc", bufs=1)
                    nc.vector.tensor_copy(mlc[:],
                                          ml[:, :, :].rearrange("p j c -> p c j"))
                    mhf = mhc[:, :, :].rearrange("p c j -> p (c j)")
                    mlf = mlc[:, :, :].rearrange("p c j -> p (c j)")
                    if True:
                        t_idx = chunk
                        for cg in range(32):  # 8-col groups in the tile-col
                            base = 128 * cg
                            nc.tensor.matmul(
                                hpsum[t_idx], mhf[:, base:base + 128],
                                mlf[:, base:base + 128],
                                start=first[t_idx],
                                stop=(b == 1 and cg == 31))
                            first[t_idx] = False

            # ---------- phase 3: per-tile hist finalize -> Hl [8, 256] -----
            Hl = pers.tile([8, 256], F32, tag="Hl")
            for t in range(8):
                hs = tmp.tile([128, 128], F32, tag="tmphs", bufs=1)
                nc.vector.tensor_copy(hs[:], hpsum[t])
                if dbg and t == 0:
                    nc.sync.dma_start(dbg_hs[:, :], hs[:])
                red2 = tmp.tile([16, 8, 16], F32, tag="tmpred2", bufs=1)
                for c in range(8):
                    nc.sync.dma_start(
                        red2[:, c, :],
                        hs[16 * c:16 * (c + 1), 16 * c:16 * (c + 1)])
                h16s = tmp.tile([16, 16], F32, tag="tmph16", bufs=1)
                nc.vector.tensor_reduce(
                    h16s[:], red2[:, :, :].rearrange("p c l -> p l c"),
                    axis=AX.X, op=AL.add)
                nc.sync.dma_start(
                    Hl[t:t + 1, :].rearrange("o (h l) -> o h l", h=16), h16s[:])

            # ---------- phase 4: LUT math on [8, 256] ----------
            ex = tmp.tile([8, 256], F32, tag="tmpl", bufs=1)
            nc.vector.tensor_scalar(ex[:], Hl[:], CLIPV, 0.0, AL.subtract, AL.max)
            exs = tmp.tile([8, 1], F32, tag="tmpl1", bufs=1)
            nc.vector.tensor_reduce(exs[:], ex[:], axis=AX.X, op=AL.add)
            nc.vector.tensor_scalar(exs[:], exs[:], 1.0 / NB, None, AL.mult)
            hf = tmp.tile([8, 256], F32, tag="tmpl2", bufs=1)
            nc.vector.tensor_scalar(hf[:], Hl[:], CLIPV, exs[:], AL.min, AL.add)
            zed = tmp.tile([8, 256], F32, tag="tmpl3", bufs=1)
            nc.vector.memset(zed[:], 0.0)
            cdf = tmp.tile([8, 256], F32, tag="tmpl4", bufs=1)
            nc.vector.tensor_tensor_scan(cdf[:], hf[:], zed[:], 0.0, AL.add, AL.add)
            nc.vector.tensor_scalar(cdf[:], cdf[:], LSCALE, 0.5, AL.mult, AL.add)
            luti = tmp.tile([8, 256], I32, tag="tmpl5", bufs=1)
            nc.vector.tensor_copy(luti[:], cdf[:])          # RNE round
            lutf = pers.tile([8, 256], F32, tag="lutf")
            nc.vector.tensor_copy(lutf[:], luti[:])
            nc.vector.tensor_scalar(lutf[:], lutf[:], 255.0, None, AL.min)

            # ---------- phase 5: exchange ----------
            lut_d = dr.tile([8, 256], F32)
            nc.sync.dma_start(lut_d.opt(), lutf[:])
            if use_collective:
                all_d = dr.tile([8 * n_cores, 256], F32)
                nc.gpsimd.collective_compute(
                    "AllGather", AL.bypass,
                    replica_groups=[list(range(n_cores))],
                    ins=[lut_d.opt()], outs=[all_d.opt()])
                lutsrc = all_d
            else:
                lutsrc = lut_d

            # ---------- phase 6: row-lut broadcast + pair-packed tables ----
            sel_sb = pers.tile([1, 3], I32, tag="selsb")
            nc.sync.dma_start(sel_sb[:], sel_in[:])
            lut_flat = lutsrc.opt().rearrange("a b -> (a b)")
            WT = []
            with nc.gpsimd.register("selr") as selreg:
                for i in range(3):
                    nc.gpsimd.reg_load(selreg, sel_sb[:1, i:i + 1])
                    off = nc.gpsimd.snap(selreg)
                    wt = tmp.tile([128, 2048], F32, tag="tmp", name=f"wt{i}")
                    nc.gpsimd.dma_start(
                        wt[:], lut_flat[bass.ds(off, 2048)].partition_broadcast(128))
                    WT.append(wt)
            wy_t, VTP = [], []
            for b in range(2):
                w = pers.tile([128, 1], F32, tag=f"wyt{b}")
                nc.sync.dma_start(w[:],
                                  wy_in[b, :].rearrange("(p o) -> p o", o=1))
                wy_t.append(w)
                vtp = pers.tile([128, 2048], U16, tag=f"vtp{b}")
                nc.vector.scalar_tensor_tensor(vtp[:], WT[b + 1][:], 256.0,
                                               WT[b][:], AL.mult, AL.add)
                VTP.append(vtp)

            wxt = pers.tile([128, 2048], F32, tag="wxt")
            nc.sync.dma_start(wxt[:], wx_in[0:1, :].partition_broadcast(128))

            # ---------- phase 7: the apply (masked accumulate) ----------
            iota = pers.tile([128, 256], F32, tag="iota")
            nc.gpsimd.iota(iota[:], pattern=[[1, 256]], base=0,
                           channel_multiplier=0,
                           allow_small_or_imprecise_dtypes=True)
            accA = [pers.tile([128, 2048], U16, tag=f"accA{b}", name=f"accA{b}")
                    for b in range(2)]
            accB = [pers.tile([128, 2048], U16, tag=f"accB{b}", name=f"accB{b}") for b in range(2)]
            for b in range(2):
                nc.vector.memset(accA[b][:], 0)
                nc.vector.memset(accB[b][:], 0)
            mask = [pers.tile([128, 2048], BF16, tag=f"msk{b}", name=f"msk{b}") for b in range(2)]

            def wbody(w):
                for b in range(2):
                    nc.vector.tensor_scalar(mask[b][:], keep[b]["vb"][:],
                                            iota[:, bass.ds(w, 1)], None,
                                            AL.is_equal)
                    for (lo, hi, t0, t1) in SPANS:
                        nc.vector.scalar_tensor_tensor(
                            accA[b][:, lo:hi], mask[b][:, lo:hi],
                            VTP[b][:, bass.ds(w + 256 * t0, 1)],
                            accA[b][:, lo:hi], AL.mult, AL.add)
                        nc.vector.scalar_tensor_tensor(
                            accB[b][:, lo:hi], mask[b][:, lo:hi],
                            VTP[b][:, bass.ds(w + 256 * t1, 1)],
                            accB[b][:, lo:hi], AL.mult, AL.add)

            tc.For_i_unrolled(0, n_w, 1, wbody, max_unroll=8)

            if dbg:
                nc.sync.dma_start(dbg_lq[0], keep[0]["Lq"][:])
                nc.sync.dma_start(dbg_lq[1], keep[1]["Lq"][:])
                nc.sync.dma_start(dbg_hl[:, :], Hl[:])
                nc.sync.dma_start(dbg_lut[:, :], lutf[:])
                _da = tmp.tile([128, 2048], I32, tag="tmp", name="_da")
                nc.vector.tensor_copy(_da[:], accA[0][:])
                nc.sync.dma_start(dbg_acc[0], _da[:])
                _db = tmp.tile([128, 2048], I32, tag="tmp", name="_db")
                nc.vector.tensor_copy(_db[:], accB[0][:])
                nc.sync.dma_start(dbg_acc[1], _db[:])
                _dv = tmp.tile([128, 2048], I32, tag="tmp", name="_dv")
                nc.vector.tensor_copy(_dv[:], VTP[0][:])
                nc.sync.dma_start(dbg_vtp[:, :], _dv[:])

            # ---------- phase 8: unpack, blend, color backward, out --------
            for b in range(2):
                ZV = []
                for acc in (accA[b], accB[b]):
                    ai = tmp.tile([128, 2048], I32, tag="tmp")
                    nc.vector.tensor_copy(ai[:], acc[:])
                    e1i = tmp.tile([128, 2048], I32, tag="tmp")
                    nc.vector.tensor_scalar(e1i[:], ai[:], 8, None,
                                            AL.logical_shift_right)
                    e1 = tmp.tile([128, 2048], F32, tag="tmp")
                    nc.vector.tensor_copy(e1[:], e1i[:])
                    nc.vector.tensor_scalar(ai[:], ai[:], 255, None,
                                            AL.bitwise_and)
                    e0 = tmp.tile([128, 2048], F32, tag="tmp")
                    nc.vector.tensor_copy(e0[:], ai[:])
                    # vertical: V = top + wy*(bot - top) ; top = e0, bot = e1
                    nc.vector.tensor_tensor(e1[:], e1[:], e0[:], AL.subtract)
                    v = tmp.tile([128, 2048], F32, tag="tmp")
                    nc.vector.scalar_tensor_tensor(v[:], e1[:], wy_t[b][:], e0[:],
                                                   AL.mult, AL.add)
                    ZV.append(v)
                va, vb_ = ZV
                nc.vector.tensor_tensor(vb_[:], vb_[:], va[:], AL.subtract)
                nc.vector.tensor_tensor(vb_[:], vb_[:], wxt[:], AL.mult)
                L255 = tmp.tile([128, 2048], F32, tag="tmp")
                nc.vector.tensor_tensor(L255[:], va[:], vb_[:], AL.add)
                fy2 = tmp.tile([128, 2048], F32, tag="tmp")
                nc.vector.tensor_scalar(fy2[:], L255[:],
                                        100.0 / (255.0 * 116.0), 16.0 / 116.0,
                                        AL.mult, AL.add)
                fx2 = tmp.tile([128, 2048], F32, tag="tmp")
                nc.vector.tensor_tensor(fx2[:], fy2[:], keep[b]["ap"][:], AL.add)
                fz2 = tmp.tile([128, 2048], F32, tag="tmp")
                nc.vector.tensor_tensor(fz2[:], fy2[:], keep[b]["bp"][:],
                                        AL.subtract)
                xyz2 = []
                for f2 in (fx2, fy2, fz2):
                    sq = tmp.tile([128, 2048], F32, tag="tmp")
                    nc.scalar.activation(sq[:], f2[:], ACT.Square)
                    cu = tmp.tile([128, 2048], F32, tag="tmp")
                    nc.vector.tensor_tensor(cu[:], sq[:], f2[:], AL.mult)
                    li = tmp.tile([128, 2048], F32, tag="tmp")
                    nc.vector.tensor_scalar(li[:], f2[:], 1.0 / 7.787,
                                            -(16.0 / 116.0) / 7.787,
                                            AL.mult, AL.add)
                    mk = tmp.tile([128, 2048], I8, tag="tmp8", bufs=2)
                    nc.vector.tensor_scalar(mk[:], cu[:], EPS, None, AL.is_gt)
                    nc.vector.select(f2[:], mk[:], cu[:], li[:])
                    xyz2.append(f2)
                for ch in range(3):
                    o = tmp.tile([128, 2048], F32, tag="tmp")
                    nc.vector.tensor_scalar(o[:], xyz2[0][:], float(M_BWD[ch][0]),
                                            None, AL.mult)
                    nc.vector.scalar_tensor_tensor(o[:], xyz2[1][:],
                                                   float(M_BWD[ch][1]), o[:],
                                                   AL.mult, AL.add)
                    nc.vector.scalar_tensor_tensor(o[:], xyz2[2][:],
                                                   float(M_BWD[ch][2]), o[:],
                                                   AL.mult, AL.add)
                    nc.vector.tensor_scalar(o[:], o[:], 0.0, 1.0, AL.max, AL.min)
                    nc.sync.dma_start(out_t[ch, 128 * b:128 * (b + 1), :], o[:])

    nc.compile()
    es.close()
    return nc


def host_inputs(core: int, x: np.ndarray, n_cores: int = NCORES):
    c = core
    rgb = np.ascontiguousarray(x[:, TRW * c:TRW * (c + 1), :]).astype(np.float32)
    u = np.arange(256, dtype=np.float64)
    wy = np.where(u < 128, 0.5 + (u + 0.5) / 256.0, (u + 0.5) / 256.0 - 0.5)
    wy = wy.astype(np.float32).reshape(2, 128)
    xcol = np.arange(W, dtype=np.float64)
    fx = np.clip((xcol + 0.5) / 256.0 - 0.5, 0.0, 7.0)
    wx = (fx - np.floor(fx)).astype(np.float32).reshape(1, W)
    cp, cn = max(c - 1, 0), min(c + 1, n_cores - 1)
    sel = np.array([[cp * 2048, c * 2048, cn * 2048]], np.int32)
    return {"rgb": rgb, "wy": wy, "wx": wx, "sel": sel}


_NC_CACHE = {}


def kernel(x: np.ndarray) -> np.ndarray:
    x = np.asarray(x, dtype=np.float32)
    if "k8" not in _NC_CACHE:
        _NC_CACHE["k8"] = build_kernel(NCORES, use_collective=True)
    nc = _NC_CACHE["k8"]
    in_maps = [host_inputs(c, x) for c in range(NCORES)]
    res = run_bass_kernel_spmd(nc, in_maps, core_ids=list(range(NCORES)))
    out = np.concatenate([r["out"] for r in res.results], axis=1)
    return out.astype(np.float32)
